# revision 6
# baseline (speedup 1.0000x reference)
"""Trainium2 Bass kernel for nn_DenseProduct (num_factors=2).

Computes, for input x of shape (128, 16, 64, 32) f32:
    out[s, d, b, i*32+j] = x[2s, d, b, i] + x[2s+1, d, b, j]
with output shape (64, 16, 64, 1024) f32.

Sharding: scope axis (dim 0) across 8 NeuronCores — core c handles output
scopes [8c, 8c+8).

The rel-err budget (2e-2) admits bf16, which halves the HBM write traffic
(the kernel is output-write bound: 256 MiB f32 -> 128 MiB bf16 total).

DVE 2x_1p perf mode requires every operand's innermost AP dim to be
stride +-1 with a 2-byte dtype. A plain broadcast outer-sum
    out[p,(bl,i,j)] = a[p,(bl,i)] + c[p,(bl,j)]
always leaves one operand with innermost stride 0. Instead iterate the
32x32 tile along wrap-around diagonals: with c doubled (c2 = [c|c]),
    out2[p, (bl, dd, t)] = a[p, (bl, t)] + c2[p, (bl, dd + t)]
every operand is innermost stride-1 (the stride-0 / stride-1-overlap dims
move to the middle), so the single tensor_tensor per scope runs at
2 elem/cycle/lane. out2 holds out_std[i=t, j=(dd+t)%32]; the host undoes
the diagonal permutation with one gather on the last axis.

Scope 0 (the latency-critical one) arrives host-packed as rows of
[a(32) | c(32) | c(32)] so its first TT needs no on-device prep; scopes
1-7 arrive compact ([a|c], k=64) and a small DVE copy builds each
doubled-c window, trading ~2.6us of idle DVE time for ~0.4 MiB of DMA.
"""

import numpy as np
import ml_dtypes

_S_IN = 128        # total input scopes
_NF = 2            # num_factors (hardcoded)
_S_OUT = _S_IN // _NF
_D = 16
_B = 64
_N = 32
_N_CORES = 8
_S_LOC = _S_OUT // _N_CORES    # 8 output scopes per core
_P = 128
_BH = 8
_BL = 8
_K96 = 3 * _N                  # scope-0 row: a | c | c
_K64 = 2 * _N                  # scopes 1-7 row: a | c
_FREE96 = _BL * _K96           # 768
_FREE64 = _BL * _K64           # 512
_FREE_OUT = _BL * _N * _N      # 8192 per partition per scope

_CACHE = {}
LAST_RESULTS = None  # BassKernelResults of the most recent run (for profiling)


def _diag_unperm():
    """index vector g: out_std[..., k] = out2[..., g[k]]."""
    k = np.arange(_N * _N)
    i = k // _N
    j = k % _N
    dd = (j - i) % _N
    return (dd * _N + i).astype(np.int64)


def _build_bass():
    import concourse.bacc as bacc
    import concourse.mybir as mybir
    from concourse.ap import AP
    from concourse.tile import TileContext

    nc = bacc.Bacc("TRN2", target_bir_lowering=False, debug=False,
                   num_devices=_N_CORES)
    x0 = nc.dram_tensor("x0", [_D, _B, _K96], mybir.dt.bfloat16,
                        kind="ExternalInput").ap()
    xs = nc.dram_tensor("xs", [_S_LOC - 1, _D, _B, _K64], mybir.dt.bfloat16,
                        kind="ExternalInput").ap()
    out = nc.dram_tensor("out", [_S_LOC, _P, _FREE_OUT], mybir.dt.bfloat16,
                         kind="ExternalOutput").ap()

    with TileContext(nc) as tc:
        with tc.tile_pool(name="head", bufs=1) as head_pool, \
             tc.tile_pool(name="in0", bufs=1) as in0_pool, \
             tc.tile_pool(name="in1", bufs=1) as in1_pool, \
             tc.tile_pool(name="inrest", bufs=1) as rest_pool, \
             tc.tile_pool(name="c2", bufs=_S_LOC - 1) as c2_pool, \
             tc.tile_pool(name="outp", bufs=4) as out_pool:
            xr0 = x0.rearrange("d (bh bl) k -> (d bh) (bl k)", bh=_BH)
            xsr = xs.rearrange("s d (bh bl) k -> (d bh) s (bl k)", bh=_BH)
            # Input issue plan: each DMA_DIRECT2D occupies its issuing engine
            # ~0.6us, so spread issues over both engines. The tiny head strip
            # (scope 0, bl=0: 24 KB) lands first so the first TT piece (and
            # with it the output stream) starts as early as possible.
            head = head_pool.tile([_P, _K96], mybir.dt.bfloat16)
            nc.sync.dma_start(out=head[:, :], in_=xr0[:, 0:_K96])
            t0 = in0_pool.tile([_P, _FREE96], mybir.dt.bfloat16)
            nc.scalar.dma_start(out=t0[:, :], in_=xr0)
            t1 = in1_pool.tile([_P, _FREE64], mybir.dt.bfloat16)
            nc.sync.dma_start(out=t1[:, :], in_=xsr[:, 0])
            trest = rest_pool.tile([_P, 6 * _FREE64], mybir.dt.bfloat16)
            nc.scalar.dma_start(
                out=trest[:, :].rearrange("p (s f) -> p s f", s=6),
                in_=xsr[:, 1:_S_LOC - 1])

            def in_src(s):
                if s == 1:
                    return t1[:, :]
                return trest[:, (s - 2) * _FREE64:(s - 1) * _FREE64]

            ndma = 0
            for s in range(_S_LOC):
                # (bl0, w) pieces: scope 0 ramps up so the first output DMA
                # issues early; the last scope tapers so the tail drain after
                # the final TT is only 512 KiB.
                if s == 0:
                    pieces = [(0, 1), (1, 1), (2, 2), (4, 4)]
                elif s in (1, 2):
                    pieces = [(0, 4), (4, 4)]
                elif s == _S_LOC - 1:
                    pieces = [(0, 3), (3, 3), (6, 2)]
                else:
                    pieces = [(0, _BL)]
                ot = out_pool.tile([_P, _FREE_OUT], mybir.dt.bfloat16)
                if s > 0:
                    # build the doubled-c window tile: c2[bl, r*32+j] = c[bl, j]
                    src = in_src(s)
                    c2t = c2_pool.tile([_P, _FREE64], mybir.dt.bfloat16)
                    spdim = list(src.ap[0])
                    dpdim = list(c2t[:, :].ap[0])
                    csrc = AP(src.tensor, src.offset + _N,
                              [spdim, [_K64, _BL], [0, 2], [1, _N]])
                    cdst = AP(c2t[:, :].tensor, c2t[:, :].offset,
                              [dpdim, [_K64, _BL], [_N, 2], [1, _N]])
                    nc.vector.tensor_copy(cdst, csrc)
                for bl0, w in pieces:
                    # out2[p, bl, dd, t] = a[p, bl, t] + c2[p, bl, dd + t]
                    if s == 0:
                        src = head[:, :] if bl0 == 0 else t0[:, :]
                        ka = _K96
                        aoff = 0 if bl0 == 0 else bl0 * _K96
                        pa = list(src.ap[0])
                        a = AP(src.tensor, src.offset + aoff,
                               [pa, [_K96, w], [0, _N], [1, _N]])
                        c2 = AP(src.tensor, src.offset + aoff + _N,
                                [pa, [_K96, w], [1, _N], [1, _N]])
                    else:
                        src = in_src(s)
                        pa = list(src.ap[0])
                        a = AP(src.tensor, src.offset + bl0 * _K64,
                               [pa, [_K64, w], [0, _N], [1, _N]])
                        pc = list(c2t[:, :].ap[0])
                        c2 = AP(c2t[:, :].tensor,
                                c2t[:, :].offset + bl0 * _K64,
                                [pc, [_K64, w], [1, _N], [1, _N]])
                    f0 = bl0 * _N * _N
                    sz = w * _N * _N
                    o4 = ot[:, f0:f0 + sz].rearrange(
                        "p (bl dd t) -> p bl dd t", bl=w, dd=_N)
                    nc.vector.tensor_add(o4, a, c2)
                    # Two HWDGE rings. The scalar ring starts busy with the
                    # bulk input DMAs, so the first outputs go on sync; then
                    # strict alternation hides each DMA's ~1us completion
                    # boundary under the other ring's data stream.
                    if ndma < 3:
                        eng = nc.sync
                    else:
                        eng = nc.scalar if ndma % 2 == 1 else nc.sync
                    eng.dma_start(out=out[s][:, f0:f0 + sz],
                                  in_=ot[:, f0:f0 + sz])
                    ndma += 1
    nc.compile()
    return nc


def kernel(x, num_factors):
    global LAST_RESULTS
    from concourse.bass_utils import run_bass_kernel_spmd

    x = np.asarray(x)
    assert x.shape == (_S_IN, _D, _B, _N), x.shape
    assert int(num_factors) == _NF, num_factors

    xb = x.astype(ml_dtypes.bfloat16)
    a = xb[0::2]   # [64, 16, 64, 32] factor-0 rows per output scope
    c = xb[1::2]   # factor-1 rows
    ac = np.concatenate([a, c], axis=-1)       # [64, 16, 64, 64]
    s0 = _S_LOC * np.arange(_N_CORES)          # each core's scope 0

    if "nc" not in _CACHE:
        _CACHE["nc"] = _build_bass()
        _CACHE["g"] = _diag_unperm()
    nc = _CACHE["nc"]

    in_maps = []
    for cc in range(_N_CORES):
        lo = cc * _S_LOC
        x0 = np.concatenate([a[lo], c[lo], c[lo]], axis=-1)  # [16, 64, 96]
        in_maps.append({
            "x0": np.ascontiguousarray(x0),
            "xs": np.ascontiguousarray(ac[lo + 1:lo + _S_LOC]),
        })
    res = run_bass_kernel_spmd(nc, in_maps, core_ids=list(range(_N_CORES)))
    LAST_RESULTS = res
    raw = np.concatenate(
        [np.asarray(res.results[cc]["out"]) for cc in range(_N_CORES)], axis=0)
    # [64, P=(d, bh), (bl, dd, t)] -> [64, d, bh, bl, 1024(dd,t)]
    raw = raw.reshape(_S_OUT, _D, _BH, _BL, _N * _N)
    out = raw[..., _CACHE["g"]].astype(np.float32)
    return out.reshape(_S_OUT, _D, _B, _N ** _NF)


# revision 7
# speedup vs baseline: 1.0241x; 1.0241x over previous
"""Trainium2 Bass kernel for nn_DenseProduct (num_factors=2).

Computes, for input x of shape (128, 16, 64, 32) f32:
    out[s, d, b, i*32+j] = x[2s, d, b, i] + x[2s+1, d, b, j]
with output shape (64, 16, 64, 1024) f32.

Sharding: scope axis (dim 0) across 8 NeuronCores — core c handles output
scopes [8c, 8c+8).

The rel-err budget (2e-2) admits bf16, which halves the HBM write traffic
(the kernel is output-write bound: 256 MiB f32 -> 128 MiB bf16 total).

DVE 2x_1p perf mode requires every operand's innermost AP dim to be
stride +-1 with a 2-byte dtype. A plain broadcast outer-sum
    out[p,(bl,i,j)] = a[p,(bl,i)] + c[p,(bl,j)]
always leaves one operand with innermost stride 0. Instead iterate the
32x32 tile along wrap-around diagonals: with c doubled (c2 = [c|c]),
    out2[p, (bl, dd, t)] = a[p, (bl, t)] + c2[p, (bl, dd + t)]
every operand is innermost stride-1 (the stride-0 / stride-1-overlap dims
move to the middle), so the single tensor_tensor per scope runs at
2 elem/cycle/lane. out2 holds out_std[i=t, j=(dd+t)%32]; the host undoes
the diagonal permutation with one gather on the last axis.

Scope 0 (the latency-critical one) arrives host-packed as rows of
[a(32) | c(32) | c(32)] so its first TT needs no on-device prep; scopes
1-7 arrive compact ([a|c], k=64) and a small DVE copy builds each
doubled-c window, trading ~2.6us of idle DVE time for ~0.4 MiB of DMA.
"""

import numpy as np
import ml_dtypes

_S_IN = 128        # total input scopes
_NF = 2            # num_factors (hardcoded)
_S_OUT = _S_IN // _NF
_D = 16
_B = 64
_N = 32
_N_CORES = 8
_S_LOC = _S_OUT // _N_CORES    # 8 output scopes per core
_P = 128
_BH = 8
_BL = 8
_K96 = 3 * _N                  # scope-0 row: a | c | c
_K64 = 2 * _N                  # scopes 1-7 row: a | c
_FREE96 = _BL * _K96           # 768
_FREE64 = _BL * _K64           # 512
_FREE_OUT = _BL * _N * _N      # 8192 per partition per scope

_CACHE = {}
LAST_RESULTS = None  # BassKernelResults of the most recent run (for profiling)


def _diag_unperm():
    """index vector g: out_std[..., k] = out2[..., g[k]]."""
    k = np.arange(_N * _N)
    i = k // _N
    j = k % _N
    dd = (j - i) % _N
    return (dd * _N + i).astype(np.int64)


def _build_bass():
    import concourse.bacc as bacc
    import concourse.mybir as mybir
    from concourse.ap import AP
    from concourse.tile import TileContext

    nc = bacc.Bacc("TRN2", target_bir_lowering=False, debug=False,
                   num_devices=_N_CORES)
    x0 = nc.dram_tensor("x0", [_D, _B, _K96], mybir.dt.bfloat16,
                        kind="ExternalInput").ap()
    xs = nc.dram_tensor("xs", [_S_LOC - 1, _D, _B, _K64], mybir.dt.bfloat16,
                        kind="ExternalInput").ap()
    out = nc.dram_tensor("out", [_S_LOC, _P, _FREE_OUT], mybir.dt.bfloat16,
                         kind="ExternalOutput").ap()

    with TileContext(nc) as tc:
        with tc.tile_pool(name="head", bufs=1) as head_pool, \
             tc.tile_pool(name="in0", bufs=1) as in0_pool, \
             tc.tile_pool(name="in1", bufs=1) as in1_pool, \
             tc.tile_pool(name="inrest", bufs=1) as rest_pool, \
             tc.tile_pool(name="c2", bufs=_S_LOC - 1) as c2_pool, \
             tc.tile_pool(name="outp", bufs=4) as out_pool:
            xr0 = x0.rearrange("d (bh bl) k -> (d bh) (bl k)", bh=_BH)
            xsr = xs.rearrange("s d (bh bl) k -> (d bh) s (bl k)", bh=_BH)
            # Input issue plan: each DMA_DIRECT2D occupies its issuing engine
            # ~0.6us, so spread issues over both engines. The tiny head strip
            # (scope 0, bl=0: 24 KB) lands first so the first TT piece (and
            # with it the output stream) starts as early as possible.
            head = head_pool.tile([_P, _K96], mybir.dt.bfloat16)
            nc.sync.dma_start(out=head[:, :], in_=xr0[:, 0:_K96])
            t0 = in0_pool.tile([_P, _FREE96], mybir.dt.bfloat16)
            nc.scalar.dma_start(out=t0[:, :], in_=xr0)
            t1 = in1_pool.tile([_P, _FREE64], mybir.dt.bfloat16)
            nc.sync.dma_start(out=t1[:, :], in_=xsr[:, 0])
            trest = rest_pool.tile([_P, 6 * _FREE64], mybir.dt.bfloat16)
            nc.scalar.dma_start(
                out=trest[:, :].rearrange("p (s f) -> p s f", s=6),
                in_=xsr[:, 1:_S_LOC - 1])

            def in_src(s):
                if s == 1:
                    return t1[:, :]
                return trest[:, (s - 2) * _FREE64:(s - 1) * _FREE64]

            ndma = 0
            for s in range(_S_LOC):
                # (bl0, w) pieces: scope 0 ramps up so the first output DMA
                # issues early; the last scope tapers so the tail drain after
                # the final TT is only 512 KiB.
                if s == 0:
                    pieces = [(0, 1), (1, 1), (2, 2), (4, 4)]
                elif s == _S_LOC - 1:
                    pieces = [(0, 3), (3, 3), (6, 2)]
                else:
                    # two pieces per scope: the first 1 MiB DMA issues
                    # halfway through the scope's TT, keeping the DMA
                    # stream fed (one 2 MiB DMA per 4.8us supply cadence
                    # barely matches the ~420 GB/s drain and starves on
                    # any hiccup)
                    pieces = [(0, 4), (4, 4)]
                ot = out_pool.tile([_P, _FREE_OUT], mybir.dt.bfloat16)
                if s > 0:
                    # build the doubled-c window tile: c2[bl, r*32+j] = c[bl, j]
                    src = in_src(s)
                    c2t = c2_pool.tile([_P, _FREE64], mybir.dt.bfloat16)
                    spdim = list(src.ap[0])
                    dpdim = list(c2t[:, :].ap[0])
                    csrc = AP(src.tensor, src.offset + _N,
                              [spdim, [_K64, _BL], [0, 2], [1, _N]])
                    cdst = AP(c2t[:, :].tensor, c2t[:, :].offset,
                              [dpdim, [_K64, _BL], [_N, 2], [1, _N]])
                    nc.vector.tensor_copy(cdst, csrc)
                for bl0, w in pieces:
                    # out2[p, bl, dd, t] = a[p, bl, t] + c2[p, bl, dd + t]
                    if s == 0:
                        src = head[:, :] if bl0 == 0 else t0[:, :]
                        ka = _K96
                        aoff = 0 if bl0 == 0 else bl0 * _K96
                        pa = list(src.ap[0])
                        a = AP(src.tensor, src.offset + aoff,
                               [pa, [_K96, w], [0, _N], [1, _N]])
                        c2 = AP(src.tensor, src.offset + aoff + _N,
                                [pa, [_K96, w], [1, _N], [1, _N]])
                    else:
                        src = in_src(s)
                        pa = list(src.ap[0])
                        a = AP(src.tensor, src.offset + bl0 * _K64,
                               [pa, [_K64, w], [0, _N], [1, _N]])
                        pc = list(c2t[:, :].ap[0])
                        c2 = AP(c2t[:, :].tensor,
                                c2t[:, :].offset + bl0 * _K64,
                                [pc, [_K64, w], [1, _N], [1, _N]])
                    f0 = bl0 * _N * _N
                    sz = w * _N * _N
                    o4 = ot[:, f0:f0 + sz].rearrange(
                        "p (bl dd t) -> p bl dd t", bl=w, dd=_N)
                    nc.vector.tensor_add(o4, a, c2)
                    # Two HWDGE rings. The scalar ring starts busy with the
                    # bulk input DMAs, so the first outputs go on sync; then
                    # strict alternation hides each DMA's ~1us completion
                    # boundary under the other ring's data stream.
                    if ndma < 3:
                        eng = nc.sync
                    else:
                        eng = nc.scalar if ndma % 2 == 1 else nc.sync
                    eng.dma_start(out=out[s][:, f0:f0 + sz],
                                  in_=ot[:, f0:f0 + sz])
                    ndma += 1
    nc.compile()
    return nc


def kernel(x, num_factors):
    global LAST_RESULTS
    from concourse.bass_utils import run_bass_kernel_spmd

    x = np.asarray(x)
    assert x.shape == (_S_IN, _D, _B, _N), x.shape
    assert int(num_factors) == _NF, num_factors

    xb = x.astype(ml_dtypes.bfloat16)
    a = xb[0::2]   # [64, 16, 64, 32] factor-0 rows per output scope
    c = xb[1::2]   # factor-1 rows
    ac = np.concatenate([a, c], axis=-1)       # [64, 16, 64, 64]
    s0 = _S_LOC * np.arange(_N_CORES)          # each core's scope 0

    if "nc" not in _CACHE:
        _CACHE["nc"] = _build_bass()
        _CACHE["g"] = _diag_unperm()
    nc = _CACHE["nc"]

    in_maps = []
    for cc in range(_N_CORES):
        lo = cc * _S_LOC
        x0 = np.concatenate([a[lo], c[lo], c[lo]], axis=-1)  # [16, 64, 96]
        in_maps.append({
            "x0": np.ascontiguousarray(x0),
            "xs": np.ascontiguousarray(ac[lo + 1:lo + _S_LOC]),
        })
    res = run_bass_kernel_spmd(nc, in_maps, core_ids=list(range(_N_CORES)))
    LAST_RESULTS = res
    raw = np.concatenate(
        [np.asarray(res.results[cc]["out"]) for cc in range(_N_CORES)], axis=0)
    # [64, P=(d, bh), (bl, dd, t)] -> [64, d, bh, bl, 1024(dd,t)]
    raw = raw.reshape(_S_OUT, _D, _BH, _BL, _N * _N)
    out = raw[..., _CACHE["g"]].astype(np.float32)
    return out.reshape(_S_OUT, _D, _B, _N ** _NF)
